# revision 1
# baseline (speedup 1.0000x reference)
"""Trainium2 Bass kernel for nn_CircuitBuilder (topk_masking).

Computes, for X [524288, 64] (f32), gate_weights [64, 130], output_weights
[64, 8], output_scale [8]:

    buf = [X | 0 | 1 | gate slots]
    top2[i] = top-2 of softmax(gate_weights[i, :66+i])   (data-independent
              of X; softmax is monotonic so = top-2 of masked logits)
    g_i = 1 - a*b  (continuous NAND chain, a/b gathered from buf)
    out = (gate_matrix @ output_weights) * output_scale

Strategy (pure data parallel over 8 NeuronCores, 65536 samples each):
  - The gate wiring is computed on host from gate_weights (tiny); the
    device kernel is built for that wiring.
  - Per-core layout: partition p owns 512 consecutive samples, processed
    as 2 supertiles of nsub=256 samples per partition. X tile is
    [128, nsub, 64] f32 (contiguous DMA); gate values live in a
    [128, 64, nsub] bf16 tile ("v" storage), where slot i holds
    v_i = alpha_i * (a_i*b_i) with a per-gate sign alpha chosen so each
    gate needs 1 fused DVE op (2 for gate×gate gates):
        m_i = a*b,  g_i = 1 - m_i,  v_i = alpha_i * m_i
    using scalar_tensor_tensor  out = (in0 op0 scalar) op1 in1.
  - Output: out = bias - sum_i W_i*m_i = bias + sum_i Wneg_i*v_i with
    Wneg_i = -alpha_i*W_i*scale. The v tile is rotated per 128-sample
    subtile with TensorE transposes into PSUM (packed bf16), drained to
    SBUF, then matmul'd against Wneg into [128, 8] psum chunks, bias
    added and DMA'd out.
"""

import hashlib
import sys
import types

import numpy as np
import ml_dtypes

N_SAMPLES = 524288
N_FEATURES = 64
N_GATES = 64
N_OUTPUTS = 8
BASE = N_FEATURES + 2            # 66
MAX_CONN = BASE + N_GATES        # 130
N_CORES = 8
N_LOC = N_SAMPLES // N_CORES     # 65536 samples per core
NSUB = 256                       # samples per partition per supertile
N_ST = N_LOC // (128 * NSUB)     # supertiles per core (2)


def _top2(gate_weights: np.ndarray) -> np.ndarray:
    """Top-2 connection indices per gate (matches jax.lax.top_k of the
    softmax: softmax is monotonic, top_k ties break to lower index,
    stable argsort of the negated row reproduces that)."""
    top2 = np.zeros((N_GATES, 2), dtype=np.int64)
    for i in range(N_GATES):
        row = np.asarray(gate_weights[i], dtype=np.float32).copy()
        row[BASE + i:] = -1e9
        top2[i] = np.argsort(-row, kind="stable")[:2]
    return top2


def _build_plan(gate_weights, output_weights, output_scale):
    """Host-side gate wiring -> per-gate op descriptors + output weights."""
    top2 = _top2(gate_weights)
    W = np.asarray(output_weights, dtype=np.float64)
    scale = np.asarray(output_scale, dtype=np.float64)

    ops = []          # list of dicts describing device ops per gate
    alpha = np.zeros(N_GATES, dtype=np.int64)
    for i in range(N_GATES):
        c0, c1 = int(top2[i][0]), int(top2[i][1])

        def kind(c):
            if c < N_FEATURES:
                return "x"
            if c == N_FEATURES:
                return "0"
            if c == N_FEATURES + 1:
                return "1"
            return "g"

        k0, k1 = kind(c0), kind(c1)
        # order canonically: g-operands first, then x, consts last
        pri = {"g": 0, "x": 1, "1": 2, "0": 3}
        if pri[k0] > pri[k1]:
            c0, c1, k0, k1 = c1, c0, k1, k0
        kk = k0 + k1
        if "0" in kk:
            ops.append({"op": "zero", "i": i})
            alpha[i] = 1
        elif kk == "x1":
            ops.append({"op": "copyx", "i": i, "c": c0})
            alpha[i] = 1
        elif kk == "g1":
            p = c0 - BASE
            ops.append({"op": "g1", "i": i, "p": p, "ap": int(alpha[p])})
            alpha[i] = 1
        elif kk == "xx":
            ops.append({"op": "xx", "i": i, "a": c0, "b": c1})
            alpha[i] = 1
        elif kk == "gx":
            p = c0 - BASE
            ap = int(alpha[p])
            ops.append({"op": "gx", "i": i, "p": p, "c": c1, "ap": ap})
            alpha[i] = -ap
        elif kk == "gg":
            p, q = c0 - BASE, c1 - BASE
            ap, aq = int(alpha[p]), int(alpha[q])
            if aq != -1 and ap == -1:
                p, q, ap, aq = q, p, aq, ap
            ops.append({"op": "gg", "i": i, "p": p, "q": q, "ap": ap, "aq": aq})
            alpha[i] = -ap if (aq == -1) else 1
        else:
            raise AssertionError(f"unexpected operand kinds {kk}")

    wneg = (-alpha[:, None] * W * scale[None, :]).astype(np.float64)
    bias = (W * scale[None, :]).sum(axis=0)
    # block-diagonal-by-subtile-parity projection matrix for the
    # pair-transpose scheme: row (g*2 + jj), col (jj*8 + o) = wneg[g, o]
    wneg2 = np.zeros((2 * N_GATES, 2 * N_OUTPUTS), dtype=np.float64)
    for jj in range(2):
        wneg2[jj::2, jj * N_OUTPUTS:(jj + 1) * N_OUTPUTS] = wneg
    return {
        "ops": ops,
        "alpha": alpha,
        "wneg_bf16": wneg.astype(ml_dtypes.bfloat16),
        "wneg2_bf16": wneg2.astype(ml_dtypes.bfloat16),
        "bias_f32": bias.astype(np.float32),
        "top2": top2,
    }


def _emulate_plan(plan, X):
    """Numpy emulation of the device program (bf16 v-storage) — used for
    host-side self-checks in development."""
    n = X.shape[0]
    bf = ml_dtypes.bfloat16
    V = np.zeros((N_GATES, n), dtype=bf)
    # device receives X pre-quantized to bf16
    Xc = np.asarray(X, dtype=np.float32).T.astype(bf).astype(np.float32)
    for d in plan["ops"]:
        i = d["i"]
        if d["op"] == "zero":
            V[i] = 0
        elif d["op"] == "copyx":
            V[i] = Xc[d["c"]].astype(bf)
        elif d["op"] == "g1":
            V[i] = (V[d["p"]].astype(np.float32) * (-d["ap"]) + 1.0).astype(bf)
        elif d["op"] == "xx":
            V[i] = (Xc[d["a"]] * Xc[d["b"]]).astype(bf)
        elif d["op"] == "gx":
            cp = -d["ap"]
            V[i] = ((V[d["p"]].astype(np.float32) + cp) * Xc[d["c"]]).astype(bf)
        elif d["op"] == "gg":
            vp = V[d["p"]].astype(np.float32)
            vq = V[d["q"]].astype(np.float32)
            if d["aq"] == -1 or d["ap"] == -1:
                cp = -d["ap"]
                t = ((vp + cp) * vq).astype(bf).astype(np.float32)
                V[i] = ((vp + cp) + t).astype(bf)
            else:
                t = ((vp - 1.0) * vq).astype(bf).astype(np.float32)
                V[i] = ((t + 1.0) - vp).astype(bf)
    wneg = plan["wneg_bf16"].astype(np.float32)
    out = V.astype(np.float32).T @ wneg + plan["bias_f32"][None, :]
    return out


def _build_bass_kernel(plan, n_loc=N_LOC, nsub=NSUB, sim_safe=False):
    import concourse.bacc as bacc
    import concourse.tile as tile
    import concourse.mybir as mybir
    from concourse import masks

    f32 = mybir.dt.float32
    bf16 = mybir.dt.bfloat16
    mult = mybir.AluOpType.mult
    add = mybir.AluOpType.add
    subtract = mybir.AluOpType.subtract

    n_st = n_loc // (128 * nsub)
    assert n_st * 128 * nsub == n_loc

    nc = bacc.Bacc(None, target_bir_lowering=False)
    # x pre-transposed on host to slot-major bf16:
    # xg[st, p, c, j] = X[st*128*nsub + p*nsub + j, c]
    x_d = nc.dram_tensor("xg", [n_st, 128, N_FEATURES, nsub], bf16,
                         kind="ExternalInput")
    wneg2_d = nc.dram_tensor("wneg2", [2 * N_GATES, 2 * N_OUTPUTS], bf16,
                             kind="ExternalInput")
    bias_d = nc.dram_tensor("bias48", [48, 1], f32, kind="ExternalInput")
    # transposed output: row (jj*8 + o), col = (st, pair, p); host decodes
    out_d = nc.dram_tensor("out", [2 * N_OUTPUTS, n_loc // 2], f32,
                           kind="ExternalOutput")

    outr = out_d.rearrange("r (s q pp) -> s r q pp",
                           s=n_st, q=nsub // 2, pp=128)

    with tile.TileContext(nc) as tc:
        with (
            tc.tile_pool(name="const", bufs=1) as cpool,
            tc.tile_pool(name="xp", bufs=2) as xpool,
            tc.tile_pool(name="vp", bufs=2) as vpool,
            tc.tile_pool(name="tp", bufs=4) as tpool,
            tc.tile_pool(name="vs", bufs=6) as vspool,
            tc.tile_pool(name="stg", bufs=3) as stgpool,
            tc.tile_pool(name="pt", bufs=4, space="PSUM") as ptpool,
            tc.tile_pool(name="po", bufs=2, space="PSUM") as popool,
        ):
            ident = cpool.tile([128, 128], bf16)
            masks.make_identity(nc, ident[:])
            wneg2_sb = cpool.tile([2 * N_GATES, 2 * N_OUTPUTS], bf16)
            nc.sync.dma_start(wneg2_sb[:], wneg2_d[:])
            bias_sb = cpool.tile([48, 1], f32)
            nc.sync.dma_start(bias_sb[:], bias_d[:])

            for st in range(n_st):
                # x: [p, col, pair, jj] slot-major bf16 (host-transposed);
                # v: [p, pair, (gate*2 + jj)] so each pair-block is
                # contiguous (1-free-dim transpose input)
                xt = xpool.tile([128, N_FEATURES, nsub // 2, 2], bf16)
                for k in range(2):
                    nc.sync.dma_start(
                        xt[:, k * 32:(k + 1) * 32, :, :],
                        x_d[st, :, k * 32:(k + 1) * 32, :].rearrange(
                            "p c (q t) -> p c q t", t=2))
                vt = vpool.tile([128, nsub // 2, 2 * N_GATES], bf16)

                def xcol(c):
                    return xt[:, c, :, :]

                def vslot(i):
                    return vt[:, :, 2 * i:2 * i + 2]

                for d in plan["ops"]:
                    i = d["i"]
                    o = d["op"]
                    if o == "zero":
                        nc.vector.memset(vslot(i), 0.0)
                    elif o == "copyx":
                        nc.vector.tensor_copy(vslot(i), xcol(d["c"]))
                    elif o == "g1":
                        nc.scalar.activation(
                            vslot(i), vslot(d["p"]),
                            mybir.ActivationFunctionType.Identity,
                            bias=1.0, scale=float(-d["ap"]))
                    elif o == "xx":
                        # stt form measures faster than plain TENSOR_TENSOR
                        nc.vector.scalar_tensor_tensor(
                            vslot(i), xcol(d["a"]), 1.0,
                            xcol(d["b"]), mult, mult)
                    elif o == "gx":
                        nc.vector.scalar_tensor_tensor(
                            vslot(i), vslot(d["p"]), float(-d["ap"]),
                            xcol(d["c"]), add, mult)
                    elif o == "gg":
                        t = tpool.tile([128, nsub // 2, 2], bf16, tag="ggtmp")
                        if d["aq"] == -1 or d["ap"] == -1:
                            cp = float(-d["ap"])
                            nc.vector.scalar_tensor_tensor(
                                t[:], vslot(d["p"]), cp, vslot(d["q"]),
                                add, mult)
                            nc.vector.scalar_tensor_tensor(
                                vslot(i), vslot(d["p"]), cp, t[:],
                                add, add)
                        else:
                            nc.vector.scalar_tensor_tensor(
                                t[:], vslot(d["p"]), -1.0, vslot(d["q"]),
                                add, mult)
                            nc.vector.scalar_tensor_tensor(
                                vslot(i), t[:], 1.0, vslot(d["p"]),
                                add, subtract)

                # output: transpose subtile PAIRS ([128, 64g x 2j] input ->
                # [128 rows=(g,jj), 128 samples]) and project with the
                # block-diagonal wneg2 (constant stationary).  PE lhsT/rhs
                # stay at base partition 0 (base-64 operands flap the PE
                # tile config and crash at scale); projection OUTPUTS pack
                # two groups per psum tile at partition offsets {0, 32} so
                # one fused bias+drain covers 16 pairs.  Host
                # de-interleaves the [16, n] transposed output.
                for big in range(nsub // 32):     # 16 pairs per iteration
                    stg = stgpool.tile([48, 1024], f32)
                    po = popool.tile([48, 1024], f32)
                    for g8 in range(2):           # 8 pairs per pt bank
                        pt = ptpool.tile([128, 1024], bf16)
                        for c in range(8):
                            pr = big * 16 + g8 * 8 + c
                            nc.tensor.transpose(
                                pt[:, c * 128:(c + 1) * 128],
                                vt[:, pr, :], ident[:])
                        vs = vspool.tile([128, 1024], bf16)
                        # last supertile's output phase is the pipeline
                        # tail: DVE is idle there, so share its drains
                        if st == n_st - 1 and g8 % 2 == 0:
                            nc.vector.tensor_copy(vs[:], pt[:])
                        else:
                            nc.scalar.copy(vs[:], pt[:])
                        for c in range(2):
                            nc.tensor.matmul(
                                po[32 * g8:32 * g8 + 16,
                                   c * 512:(c + 1) * 512],
                                wneg2_sb[:], vs[:, c * 512:(c + 1) * 512],
                                start=True, stop=True)
                    # drain + bias (per-partition scalar) fused, 16 pairs.
                    # The single [48, ...] op reads the unwritten psum gap
                    # rows 16-31 (never DMA'd, benign on HW); CoreSim
                    # flags uninitialized reads, so sim builds drain the
                    # two written slices instead.
                    drains = ([(stg[:], po[:], bias_sb[:, 0:1])]
                              if not sim_safe else
                              [(stg[0:16, :], po[0:16, :], bias_sb[0:16, 0:1]),
                               (stg[32:48, :], po[32:48, :],
                                bias_sb[32:48, 0:1])])
                    for sslice, pslice, bslice in drains:
                        if (big % 2 == 0) if st == n_st - 1 else False:
                            nc.vector.tensor_scalar(
                                sslice, pslice, bslice, None, add)
                        else:
                            nc.scalar.activation(
                                sslice, pslice,
                                mybir.ActivationFunctionType.Identity,
                                bias=bslice, scale=1.0)
                    for g8 in range(2):
                        nc.sync.dma_start(
                            outr[st, :,
                                 big * 16 + g8 * 8:big * 16 + g8 * 8 + 8, :],
                            stg[32 * g8:32 * g8 + 16, :])

    nc.compile()
    return nc


_CACHE = {}


def _get_compiled(gate_weights, output_weights, output_scale):
    key = hashlib.sha256(
        np.asarray(gate_weights, np.float32).tobytes()
        + np.asarray(output_weights, np.float32).tobytes()
        + np.asarray(output_scale, np.float32).tobytes()
    ).hexdigest()
    if key not in _CACHE:
        plan = _build_plan(gate_weights, output_weights, output_scale)
        nc = _build_bass_kernel(plan)
        _CACHE[key] = (plan, nc)
    return _CACHE[key]


def _decode_out(dev_out, plan, n_loc=N_LOC, nsub=NSUB):
    """[16, n_loc//2] transposed device output (bias included) ->
    [n_loc, 8]."""
    n_st = n_loc // (128 * nsub)
    o5 = np.asarray(dev_out).reshape(2, N_OUTPUTS, n_st, nsub // 2, 128)
    # [jj, o, st, pr, p] -> [st, p, pr, jj, o]
    return np.transpose(o5, (2, 4, 3, 0, 1)).reshape(n_loc, N_OUTPUTS)


def make_in_maps(X, plan, n_loc=N_LOC, nsub=NSUB, n_cores=N_CORES):
    bias16 = np.concatenate([plan["bias_f32"], plan["bias_f32"]])
    bias48 = np.zeros((48, 1), dtype=np.float32)
    bias48[0:16, 0] = bias16
    bias48[32:48, 0] = bias16
    n_st = n_loc // (128 * nsub)
    # slot-major bf16: xg[core][st, p, c, j] = X[...]
    xg = (X[:n_cores * n_loc]
          .reshape(n_cores, n_st, 128, nsub, N_FEATURES)
          .transpose(0, 1, 2, 4, 3)
          .astype(ml_dtypes.bfloat16))
    in_maps = []
    for c in range(n_cores):
        in_maps.append({
            "xg": np.ascontiguousarray(xg[c]),
            "wneg2": plan["wneg2_bf16"],
            "bias48": bias48,
        })
    return in_maps


def kernel(X, gate_weights, output_weights, output_scale):
    X = np.asarray(X, dtype=np.float32)
    plan, nc = _get_compiled(gate_weights, output_weights, output_scale)
    in_maps = make_in_maps(X, plan)

    from concourse.bass_utils import run_bass_kernel_spmd
    res = run_bass_kernel_spmd(nc, in_maps, list(range(N_CORES)))
    out = np.concatenate(
        [_decode_out(res.results[c]["out"], plan) for c in range(N_CORES)],
        axis=0)
    return out.astype(np.float32)



# revision 14
# speedup vs baseline: 1.2157x; 1.2157x over previous
"""Trainium2 Bass kernel for nn_CircuitBuilder (topk_masking).

For X [524288, 64] f32, gate_weights [64, 130], output_weights [64, 8],
output_scale [8]:

    buf = [X | 0 | 1 | gate slots]
    top2[i] = top-2 of softmax(gate_weights[i, :66+i])  (data-independent)
    g_i = 1 - a*b   (continuous NAND chain)
    out = (gate_matrix @ output_weights) * output_scale

Design (pure data parallel over 8 cores, 65536 samples each):
  - Host computes the gate wiring and compiles a per-wiring device program.
  - Per-core layout: partition p owns 512 consecutive samples, pair-major:
    vt[p, t, g, u] = gate-slot g at sample 2t+u (bf16). X arrives pre-packed
    the same way with only the used columns (possibly reordered/duplicated).
  - The gate DAG is shallow (4 levels). Per level the a*b products run as
    batched tensor_tensor ops (2x DVE mode): xx gates are chained into
    Euler trails over the column graph (one op per trail via overlapping
    A/B windows), gx gates batch over constant-stride parent runs. The
    "1-x" corrections run as one ranged tensor_scalar op (4x mode) per
    level; slots also covered by a range store 1-m and the projection
    weights absorb the affine flip per slot.
  - Output projection needs v transposed to gate-on-partition. Two paths:
      * PE path: f32-bitcast [128,128] transposes (two bf16 sample pairs
        per f32) -> PSUM -> bf16 drain (Act/Pool) -> matmul.
      * XBAR path: dma_start_transpose maps in[p, 128k+r] -> out[r, k, p];
        the pair-major inner 128-block is exactly (g,u), so the DMA engines
        transpose SBUF->SBUF with no PSUM round-trip.
  - Projection matmuls [16,1024] stack 8-deep into [128,1024] f32 PSUM
    groups -> one bf16 drain per group -> DMA out. Bias + final f32 are
    applied on host during decode.
"""

import hashlib

import numpy as np
import ml_dtypes

N_SAMPLES = 524288
N_FEATURES = 64
N_GATES = 64
N_OUTPUTS = 8
BASE = N_FEATURES + 2            # 66
MAX_CONN = BASE + N_GATES        # 130
N_CORES = 8
N_LOC = N_SAMPLES // N_CORES     # 65536 samples per core
P = 128
NT = N_LOC // (2 * P)            # 256 sample-pairs per partition

# schedule configuration
CHUNKS = [64, 64, 64, 64]        # DVE chunk sizes in pairs (sum == NT)
XB_UNITS = [1, 1, 1, 1]          # trailing 8-pair units per chunk on XBAR
N_UNITS = NT // 8                # 32 projection units of 8 pairs
# 4 units stack into one [128,1024] f32 PSUM group at partition offsets
# 32*{0..3} (PE tile positions allow only multiples of 32); rows 16..31 of
# each 32-block are unused.
N_GROUPS = N_UNITS // 4          # 8 po groups

assert sum(CHUNKS) == NT and all(c % 8 == 0 for c in CHUNKS)


def _top2(gate_weights: np.ndarray) -> np.ndarray:
    """Top-2 connection indices per gate (matches jax.lax.top_k of the
    softmax: softmax is monotonic, ties break to lower index)."""
    top2 = np.zeros((N_GATES, 2), dtype=np.int64)
    for i in range(N_GATES):
        row = np.asarray(gate_weights[i], dtype=np.float32).copy()
        row[BASE + i:] = -1e9
        top2[i] = np.argsort(-row, kind="stable")[:2]
    return top2


def _build_plan(gate_weights, output_weights, output_scale):
    top2 = _top2(gate_weights)
    W = np.asarray(output_weights, dtype=np.float64)
    scale = np.asarray(output_scale, dtype=np.float64)

    # ---- classify gates -------------------------------------------------
    def kind_of(c):
        if c < N_FEATURES:
            return "x"
        if c == N_FEATURES:
            return "0"
        if c == N_FEATURES + 1:
            return "1"
        return "g"

    gates = {}
    for i in range(N_GATES):
        c0, c1 = int(top2[i][0]), int(top2[i][1])
        k0, k1 = kind_of(c0), kind_of(c1)
        pri = {"g": 0, "x": 1, "1": 2, "0": 3}
        if pri[k0] > pri[k1]:
            c0, c1, k0, k1 = c1, c0, k1, k0
        kk = k0 + k1
        if "0" in kk:
            gates[i] = {"op": "zero"}
        elif kk == "x1":
            gates[i] = {"op": "copyx", "c": c0}
        elif kk == "11":
            gates[i] = {"op": "one"}
        elif kk == "g1":
            gates[i] = {"op": "g1", "p": c0 - BASE}
        elif kk == "xx":
            gates[i] = {"op": "xx", "a": c0, "b": c1}
        elif kk == "gx":
            gates[i] = {"op": "gx", "p": c0 - BASE, "c": c1}
        elif kk == "gg":
            gates[i] = {"op": "gg", "p": c0 - BASE, "q": c1 - BASE}
        else:
            raise AssertionError(f"unexpected operand kinds {kk}")

    def parents(i):
        d = gates[i]
        return [d[k] for k in ("p", "q") if k in d]

    level = {}
    for i in range(N_GATES):
        ps = parents(i)
        level[i] = (1 + max(level[p] for p in ps)) if ps else 0
    max_level = max(level.values())

    consumers = {i: [] for i in range(N_GATES)}
    for i in range(N_GATES):
        for p in parents(i):
            consumers[p].append(i)
    consumed = {i for i in range(N_GATES) if consumers[i]}

    lv_gates = {l: [i for i in range(N_GATES) if level[i] == l]
                for l in range(max_level + 1)}

    # ---- slot assignment -------------------------------------------------
    # L0: xx gates ordered by first consumer (so the next level's parent
    # slots ascend stride-1), then copyx, then zero/one (adjacent memsets).
    # For gg consumers, both parents share the consumer key and land
    # adjacent (tiebreak: p before q), giving dp=dq=2 runs.
    def fc_key(i):
        cons = [(level[j], j, 0 if gates[j].get("p") == i else 1)
                for j in consumers[i]]
        return (min(cons) if cons else (99, i, 0), i)

    slot_of = {}
    gate_of = {}
    s = 0
    ordkind = {"xx": 0, "copyx": 1, "zero": 2, "one": 2}
    l0 = sorted(lv_gates[0], key=lambda i: (ordkind[gates[i]["op"]], fc_key(i)))
    for i in l0:
        slot_of[i] = s
        s += 1
    # L1+: gx sorted by parent slot, then gg (by parent pair), then g1
    for l in range(1, max_level + 1):
        def key(i):
            d = gates[i]
            ps = sorted(slot_of[d[k]] for k in ("p", "q") if k in d)
            return ({"gx": 0, "gg": 1, "g1": 2}[d["op"]], ps[0], ps[-1], i)
        for i in sorted(lv_gates[l], key=key):
            slot_of[i] = s
            s += 1
    assert s == N_GATES
    for i, s2 in slot_of.items():
        gate_of[s2] = i

    # ---- column placement + ops -----------------------------------------
    colmap = []
    col_pos = {}

    def place(c, fresh=False):
        if not fresh and c in col_pos:
            return col_pos[c]
        pos = len(colmap)
        colmap.append(c)
        if c not in col_pos:
            col_pos[c] = pos
        return pos

    ops = []
    corrected = set()             # slots storing g = 1-m

    for l in range(max_level + 1):
        gl = sorted(lv_gates[l], key=lambda i: slot_of[i])

        idx = 0
        while idx < len(gl):
            i = gl[idx]
            d = gates[i]
            op = d["op"]
            if op == "zero":
                ops.append({"k": "memset", "s0": slot_of[i], "n": 1,
                            "val": 0.0})
                idx += 1
            elif op == "one":
                ops.append({"k": "memset", "s0": slot_of[i], "n": 1,
                            "val": 1.0})
                idx += 1
            elif op == "copyx":
                run = [i]
                jdx = idx + 1
                while jdx < len(gl) and gates[gl[jdx]]["op"] == "copyx":
                    run.append(gl[jdx])
                    jdx += 1
                c0 = None
                for g2 in run:
                    pos = place(gates[g2]["c"], fresh=len(run) > 1)
                    c0 = pos if c0 is None else c0
                ops.append({"k": "copyx", "s0": slot_of[run[0]],
                            "n": len(run), "c0": c0})
                idx = jdx
            elif op == "g1":
                ops.append({"k": "copyv", "s0": slot_of[i], "n": 1,
                            "p0": slot_of[d["p"]]})
                idx += 1
            elif op == "xx":
                # one batched op: A then B regions, every ref placed fresh
                run = [i]
                jdx = idx + 1
                while jdx < len(gl) and gates[gl[jdx]]["op"] == "xx":
                    run.append(gl[jdx])
                    jdx += 1
                if len(run) >= 2:
                    a0 = len(colmap)
                    for g2 in run:
                        place(gates[g2]["a"], fresh=True)
                    b0 = len(colmap)
                    for g2 in run:
                        place(gates[g2]["b"], fresh=True)
                    ops.append({"k": "tt", "s0": slot_of[run[0]], "ds": 1,
                                "n": len(run),
                                "in0": ("x", a0, 1), "in1": ("x", b0, 1)})
                else:
                    ops.append({"k": "tt", "s0": slot_of[i], "ds": 1, "n": 1,
                                "in0": ("x", place(d["a"]), 1),
                                "in1": ("x", place(d["b"]), 1)})
                idx = jdx
            elif op == "gx":
                # run: const parent stride; cols dup-placed to keep going
                run = [i]
                jdx = idx + 1
                dstride = None
                while jdx < len(gl):
                    nd = gates[gl[jdx]]
                    if nd["op"] != "gx":
                        break
                    step = slot_of[nd["p"]] - slot_of[gates[run[-1]]["p"]]
                    if step <= 0 or (dstride is not None and step != dstride):
                        break
                    dstride = step
                    run.append(gl[jdx])
                    jdx += 1
                if len(run) >= 2:
                    c0 = None
                    for g2 in run:
                        pos = place(gates[g2]["c"], fresh=True)
                        c0 = pos if c0 is None else c0
                    ops.append({"k": "tt", "s0": slot_of[run[0]], "ds": 1,
                                "n": len(run),
                                "in0": ("v", slot_of[d["p"]], dstride),
                                "in1": ("x", c0, 1)})
                    idx = jdx
                else:
                    ops.append({"k": "tt", "s0": slot_of[i], "ds": 1, "n": 1,
                                "in0": ("v", slot_of[d["p"]], 1),
                                "in1": ("x", place(d["c"]), 1)})
                    idx += 1
            elif op == "gg":
                run = [i]
                jdx = idx + 1
                dp = dq = None
                while jdx < len(gl):
                    nd = gates[gl[jdx]]
                    if nd["op"] != "gg":
                        break
                    sp = slot_of[nd["p"]] - slot_of[gates[run[-1]]["p"]]
                    sq = slot_of[nd["q"]] - slot_of[gates[run[-1]]["q"]]
                    if sp <= 0 or sq <= 0:
                        break
                    if dp is not None and (sp != dp or sq != dq):
                        break
                    dp, dq = sp, sq
                    run.append(gl[jdx])
                    jdx += 1
                if len(run) >= 2:
                    ops.append({"k": "tt", "s0": slot_of[run[0]], "ds": 1,
                                "n": len(run),
                                "in0": ("v", slot_of[d["p"]], dp),
                                "in1": ("v", slot_of[d["q"]], dq)})
                    idx = jdx
                else:
                    ops.append({"k": "tt", "s0": slot_of[i], "ds": 1, "n": 1,
                                "in0": ("v", slot_of[d["p"]], 1),
                                "in1": ("v", slot_of[d["q"]], 1)})
                    idx += 1
            else:
                raise AssertionError(op)

        # corrections: runs over consumed slots, bridging gaps <= 3
        cons = sorted(slot_of[i] for i in gl if i in consumed)
        while cons:
            lo = hi = cons.pop(0)
            while cons and cons[0] - hi <= 3:
                hi = cons.pop(0)
            ops.append({"k": "corr", "s0": lo, "n": hi - lo + 1})
            corrected.update(range(lo, hi + 1))

    # ---- projection constants -------------------------------------------
    wsig = np.zeros((N_GATES, N_OUTPUTS))
    bias = np.zeros(N_OUTPUTS)
    for s2 in range(N_GATES):
        wrow = W[gate_of[s2]] * scale
        if s2 in corrected:
            wsig[s2] = wrow
        else:
            wsig[s2] = -wrow
            bias += wrow

    # PE path rows: r = t'*64 + g
    wt = np.zeros((P, 2 * N_OUTPUTS))
    for tp in range(2):
        wt[tp * N_GATES:(tp + 1) * N_GATES,
           tp * N_OUTPUTS:(tp + 1) * N_OUTPUTS] = wsig
    # XBAR path rows: r = 2g + u
    wu = np.zeros((P, 2 * N_OUTPUTS))
    for u in range(2):
        wu[u::2, u * N_OUTPUTS:(u + 1) * N_OUTPUTS] = wsig

    return {
        "gates": gates,
        "ops": ops,
        "colmap": np.array(colmap, dtype=np.int64),
        "ncols": len(colmap),
        "slot_of": slot_of,
        "gate_of": gate_of,
        "consumed": consumed,
        "corrected": corrected,
        "level": level,
        "wt_bf16": wt.astype(ml_dtypes.bfloat16),
        "wu_bf16": wu.astype(ml_dtypes.bfloat16),
        "bias_f32": bias.astype(np.float32),
    }


def _emulate_vt(plan, xt):
    """Emulate the device gate program on sample-major xt [n, ncols] bf16.
    Returns V [n, 64] bf16 slot values."""
    bf = ml_dtypes.bfloat16
    n = xt.shape[0]
    V = np.zeros((n, N_GATES), dtype=bf)
    for op in plan["ops"]:
        k, s0, nn = op["k"], op["s0"], op["n"]
        if k == "memset":
            V[:, s0:s0 + nn] = op["val"]
        elif k == "copyx":
            V[:, s0:s0 + nn] = xt[:, op["c0"]:op["c0"] + nn]
        elif k == "copyv":
            V[:, s0:s0 + nn] = V[:, op["p0"]:op["p0"] + nn]
        elif k == "tt":
            def rd(spec):
                src, o0, ds = spec
                arr = xt if src == "x" else V
                return arr[:, o0:o0 + ds * nn:ds].astype(np.float32)
            V[:, s0:s0 + nn] = (rd(op["in0"]) * rd(op["in1"])).astype(bf)
        elif k == "corr":
            m = V[:, s0:s0 + nn].astype(np.float32)
            V[:, s0:s0 + nn] = (1.0 - m).astype(bf)
        else:
            raise AssertionError(k)
    return V


def _schedule():
    """Derive per-chunk unit lists. Unit k covers pairs [8k, 8k+8)."""
    sched = []
    u0 = 0
    for ci, cp in enumerate(CHUNKS):
        nu = cp // 8
        xb = XB_UNITS[ci]
        sched.append({
            "t0": sum(CHUNKS[:ci]),
            "tn": cp,
            "units": list(range(u0, u0 + nu)),
            "pe_units": list(range(u0, u0 + nu - xb)),
            "xb_units": list(range(u0 + nu - xb, u0 + nu)),
        })
        u0 += nu
    return sched


def _build_bass_kernel(plan):
    import concourse.bacc as bacc
    import concourse.tile as tile
    import concourse.mybir as mybir
    from concourse import masks

    f32 = mybir.dt.float32
    bf16 = mybir.dt.bfloat16
    mult = mybir.AluOpType.mult
    add = mybir.AluOpType.add

    C = plan["ncols"]
    sched = _schedule()

    nc = bacc.Bacc(None, target_bir_lowering=False)
    x_d = nc.dram_tensor("xg", [P, NT, C, 2], bf16, kind="ExternalInput")
    wt_d = nc.dram_tensor("wt", [P, 2 * N_OUTPUTS], bf16, kind="ExternalInput")
    wu_d = nc.dram_tensor("wu", [P, 2 * N_OUTPUTS], bf16, kind="ExternalInput")
    out_d = nc.dram_tensor("out", [N_GROUPS, P, 1024], bf16,
                           kind="ExternalOutput")

    with tile.TileContext(nc) as tc:
        with (
            tc.tile_pool(name="const", bufs=1) as cpool,
            tc.tile_pool(name="xp", bufs=1) as xpool,
            tc.tile_pool(name="vp", bufs=2) as vpool,
            tc.tile_pool(name="txp", bufs=2) as txpool,
            tc.tile_pool(name="vsp", bufs=4) as vspool,
            tc.tile_pool(name="stgp", bufs=2) as stgpool,
            tc.tile_pool(name="ptp", bufs=3, space="PSUM") as ptpool,
            tc.tile_pool(name="pop", bufs=2, space="PSUM") as popool,
        ):
            ident = cpool.tile([128, 128], f32)
            masks.make_identity(nc, ident[:])
            wt_sb = cpool.tile([P, 2 * N_OUTPUTS], bf16)
            nc.sync.dma_start(wt_sb[:], wt_d[:])
            wu_sb = cpool.tile([P, 2 * N_OUTPUTS], bf16)
            nc.sync.dma_start(wu_sb[:], wu_d[:])

            xt = xpool.tile([P, NT, C, 2], bf16)
            for ci, sc in enumerate(sched):
                t0, tn = sc["t0"], sc["tn"]
                nc.sync.dma_start(xt[:, t0:t0 + tn, :, :],
                                  x_d[:, t0:t0 + tn, :, :])

            po_tiles = {}
            po_filled = {}
            ndrain = 0

            for ci, sc in enumerate(sched):
                t0, tn = sc["t0"], sc["tn"]
                vt = vpool.tile([P, tn, N_GATES, 2], bf16, tag="vt")

                def vsl(s0, ds, n):
                    if ds == 1:
                        return vt[:, :, s0:s0 + n, :]
                    return vt[:, :, s0:s0 + ds * n:ds, :]

                def xsl(c0, dc, n):
                    if dc == 1:
                        return xt[:, t0:t0 + tn, c0:c0 + n, :]
                    return xt[:, t0:t0 + tn, c0:c0 + dc * n:dc, :]

                for op in plan["ops"]:
                    k, s0, n = op["k"], op["s0"], op["n"]
                    if k == "memset":
                        nc.gpsimd.memset(vsl(s0, 1, n), op["val"])
                    elif k == "copyx":
                        nc.vector.tensor_copy(vsl(s0, 1, n),
                                              xsl(op["c0"], 1, n))
                    elif k == "copyv":
                        nc.vector.tensor_copy(vsl(s0, 1, n),
                                              vsl(op["p0"], 1, n))
                    elif k == "tt":
                        def rd(spec):
                            src, o0, ds = spec
                            return (xsl(o0, ds, n) if src == "x"
                                    else vsl(o0, ds, n))
                        nc.vector.tensor_tensor(
                            vsl(s0, op["ds"], n), rd(op["in0"]),
                            rd(op["in1"]), mult)
                    elif k == "corr":
                        nc.vector.tensor_scalar(
                            vsl(s0, 1, n), vsl(s0, 1, n), -1.0, 1.0,
                            mult, add)
                    else:
                        raise AssertionError(k)

                # ---- output phase ----
                if sc["xb_units"]:
                    xb_p0 = (sc["xb_units"][0] * 8 - t0)   # local pair
                    xb_np = len(sc["xb_units"]) * 8
                    txt = txpool.tile([P, xb_np, P], bf16, tag="txt")
                    nc.sync.dma_start_transpose(
                        txt[:], vt[:, xb_p0:xb_p0 + xb_np, :, :])

                def get_po(k):
                    g = k // 4
                    if g not in po_tiles:
                        po_tiles[g] = popool.tile([P, 1024], f32, name="po",
                                                  tag="po")
                        po_filled[g] = 0
                    return po_tiles[g]

                vb = vt[:].bitcast(f32).rearrange("p t g z -> p (t g z)")
                for k in sc["pe_units"]:
                    po = get_po(k)
                    pt = ptpool.tile([P, 512], f32, tag="pt")
                    for kk in range(4):
                        k2 = (8 * k - t0) // 2 + kk     # local pair-pair
                        nc.tensor.transpose(
                            pt[:, 128 * kk:128 * (kk + 1)],
                            vb[:, 128 * k2:128 * (k2 + 1)], ident[:])
                    vs = vspool.tile([P, 1024], bf16, tag="vs")
                    # GPSIMD cannot access PSUM; drains go Act/Act/DVE
                    if ndrain % 3 < 2:
                        nc.scalar.copy(vs[:], pt[:].bitcast(bf16))
                    else:
                        nc.vector.tensor_copy(vs[:], pt[:].bitcast(bf16))
                    ndrain += 1
                    m = k % 4
                    for h in range(2):
                        nc.tensor.matmul(
                            po[32 * m:32 * m + 16, 512 * h:512 * h + 512],
                            wt_sb[:], vs[:, 512 * h:512 * h + 512],
                            start=True, stop=True, tile_position=(0, 32 * m))
                    po_filled[k // 4] += 1

                for k in sc["xb_units"]:
                    po = get_po(k)
                    lp = 8 * k - sc["xb_units"][0] * 8
                    m = k % 4
                    for h in range(2):
                        rhs = txt[:, lp + 4 * h:lp + 4 * h + 4, :]
                        nc.tensor.matmul(
                            po[32 * m:32 * m + 16, 512 * h:512 * h + 512],
                            wu_sb[:], rhs.rearrange("r k p -> r (k p)"),
                            start=True, stop=True, tile_position=(0, 32 * m))
                    po_filled[k // 4] += 1

                for g in sorted(po_tiles):
                    if po_filled[g] == 4:
                        stg = stgpool.tile([P, 1024], bf16, tag="stg")
                        nc.scalar.copy(stg[:], po_tiles[g][:])
                        nc.sync.dma_start(out_d[g], stg[:])
                        del po_tiles[g]
                        po_filled[g] = -1

    nc.compile()
    return nc


def _decode_idx():
    """dst[g, r, q] -> flat index into [N_LOC, 8]."""
    sched = _schedule()
    is_xb = {}
    xb_base = {}
    for sc in sched:
        for k in sc["pe_units"]:
            is_xb[k] = False
        for k in sc["xb_units"]:
            is_xb[k] = True
            xb_base[k] = sc["xb_units"][0]
    dst = np.full((N_GROUPS, P, 1024), -1, dtype=np.int64)
    for k in range(N_UNITS):
        g, m = k // 4, k % 4
        for rr in range(16):
            half, o = rr // N_OUTPUTS, rr % N_OUTPUTS
            r = 32 * m + rr
            if not is_xb[k]:
                kk = np.arange(4)[:, None, None]
                p_ = np.arange(P)[None, :, None]
                u = np.arange(2)[None, None, :]
                q = (kk * 256 + p_ * 2 + u).reshape(-1)
                j = (2 * (8 * k + 2 * kk + half) + u + 0 * p_).reshape(-1)
                p_f = (0 * kk + p_ + 0 * u).reshape(-1)
            else:
                tl = np.arange(8)[:, None]
                p_ = np.arange(P)[None, :]
                q = (tl * 128 + p_).reshape(-1)
                j = (2 * (8 * k + tl) + half + 0 * p_).reshape(-1)
                p_f = (0 * tl + p_).reshape(-1)
            dst[g, r, q] = (p_f * 512 + j) * N_OUTPUTS + o
    return dst


_DST = None


def _decode_out(dev_out, plan):
    """[N_GROUPS, P, 1024] bf16 device output -> [N_LOC, 8] f32 w/ bias."""
    global _DST
    if _DST is None:
        _DST = _decode_idx()
    flat = np.empty(N_LOC * N_OUTPUTS, dtype=np.float32)
    mask = _DST.reshape(-1) >= 0
    flat[_DST.reshape(-1)[mask]] = np.asarray(dev_out).astype(
        np.float32).reshape(-1)[mask]
    out = flat.reshape(N_LOC, N_OUTPUTS)
    out += plan["bias_f32"][None, :]
    return out


def make_in_maps(X, plan):
    colmap = plan["colmap"]
    # xg[core][p, t, c, u] = X[core*N_LOC + p*512 + 2t + u, colmap[c]]
    arr = np.asarray(X, dtype=np.float32).reshape(N_CORES, P, NT, 2, N_FEATURES)
    xg = arr[..., colmap].transpose(0, 1, 2, 4, 3).astype(ml_dtypes.bfloat16)
    in_maps = []
    for c in range(N_CORES):
        in_maps.append({
            "xg": np.ascontiguousarray(xg[c]),
            "wt": plan["wt_bf16"],
            "wu": plan["wu_bf16"],
        })
    return in_maps


_CACHE = {}


def _get_compiled(gate_weights, output_weights, output_scale):
    key = hashlib.sha256(
        np.asarray(gate_weights, np.float32).tobytes()
        + np.asarray(output_weights, np.float32).tobytes()
        + np.asarray(output_scale, np.float32).tobytes()
    ).hexdigest()
    if key not in _CACHE:
        plan = _build_plan(gate_weights, output_weights, output_scale)
        nc = _build_bass_kernel(plan)
        _CACHE[key] = (plan, nc)
    return _CACHE[key]


def kernel(X, gate_weights, output_weights, output_scale):
    X = np.asarray(X, dtype=np.float32)
    plan, nc = _get_compiled(gate_weights, output_weights, output_scale)
    in_maps = make_in_maps(X, plan)

    from concourse.bass_utils import run_bass_kernel_spmd
    res = run_bass_kernel_spmd(nc, in_maps, list(range(N_CORES)))
    out = np.concatenate(
        [_decode_out(res.results[c]["out"], plan) for c in range(N_CORES)],
        axis=0)
    return out.astype(np.float32)


# revision 15
# speedup vs baseline: 1.2444x; 1.0236x over previous
"""Trainium2 Bass kernel for nn_CircuitBuilder (topk_masking).

For X [524288, 64] f32, gate_weights [64, 130], output_weights [64, 8],
output_scale [8]:

    buf = [X | 0 | 1 | gate slots]
    top2[i] = top-2 of softmax(gate_weights[i, :66+i])  (data-independent)
    g_i = 1 - a*b   (continuous NAND chain)
    out = (gate_matrix @ output_weights) * output_scale

Design (pure data parallel over 8 cores, 65536 samples each):
  - Host computes the gate wiring and compiles a per-wiring device program.
  - Per-core layout: partition p owns 512 consecutive samples, pair-major:
    vt[p, t, g, u] = gate-slot g at sample 2t+u (bf16). X arrives pre-packed
    the same way with only the used columns (possibly reordered/duplicated).
  - The gate DAG is shallow (4 levels). Per level the a*b products run as
    batched tensor_tensor ops (2x DVE mode): xx gates are chained into
    Euler trails over the column graph (one op per trail via overlapping
    A/B windows), gx gates batch over constant-stride parent runs. The
    "1-x" corrections run as one ranged tensor_scalar op (4x mode) per
    level; slots also covered by a range store 1-m and the projection
    weights absorb the affine flip per slot.
  - Output projection needs v transposed to gate-on-partition. Two paths:
      * PE path: f32-bitcast [128,128] transposes (two bf16 sample pairs
        per f32) -> PSUM -> bf16 drain (Act/Pool) -> matmul.
      * XBAR path: dma_start_transpose maps in[p, 128k+r] -> out[r, k, p];
        the pair-major inner 128-block is exactly (g,u), so the DMA engines
        transpose SBUF->SBUF with no PSUM round-trip.
  - Projection matmuls [16,1024] stack 8-deep into [128,1024] f32 PSUM
    groups -> one bf16 drain per group -> DMA out. Bias + final f32 are
    applied on host during decode.
"""

import hashlib

import numpy as np
import ml_dtypes

N_SAMPLES = 524288
N_FEATURES = 64
N_GATES = 64
N_OUTPUTS = 8
BASE = N_FEATURES + 2            # 66
MAX_CONN = BASE + N_GATES        # 130
N_CORES = 8
N_LOC = N_SAMPLES // N_CORES     # 65536 samples per core
P = 128
NT = N_LOC // (2 * P)            # 256 sample-pairs per partition

# schedule configuration
CHUNKS = [32, 64, 80, 80]        # DVE chunk sizes in pairs (sum == NT)
XB_UNITS = [2, 4, 5, 5]          # trailing 8-pair units per chunk on XBAR
IN_SLICES = 8                    # input DMA granularity (slices of NT)
N_UNITS = NT // 8                # 32 projection units of 8 pairs
# 4 units stack into one [128,1024] f32 PSUM group at partition offsets
# 32*{0..3} (PE tile positions allow only multiples of 32); rows 16..31 of
# each 32-block are unused.
N_GROUPS = N_UNITS // 4          # 8 po groups

assert sum(CHUNKS) == NT and all(c % 8 == 0 for c in CHUNKS)


def _top2(gate_weights: np.ndarray) -> np.ndarray:
    """Top-2 connection indices per gate (matches jax.lax.top_k of the
    softmax: softmax is monotonic, ties break to lower index)."""
    top2 = np.zeros((N_GATES, 2), dtype=np.int64)
    for i in range(N_GATES):
        row = np.asarray(gate_weights[i], dtype=np.float32).copy()
        row[BASE + i:] = -1e9
        top2[i] = np.argsort(-row, kind="stable")[:2]
    return top2


def _build_plan(gate_weights, output_weights, output_scale):
    top2 = _top2(gate_weights)
    W = np.asarray(output_weights, dtype=np.float64)
    scale = np.asarray(output_scale, dtype=np.float64)

    # ---- classify gates -------------------------------------------------
    def kind_of(c):
        if c < N_FEATURES:
            return "x"
        if c == N_FEATURES:
            return "0"
        if c == N_FEATURES + 1:
            return "1"
        return "g"

    gates = {}
    for i in range(N_GATES):
        c0, c1 = int(top2[i][0]), int(top2[i][1])
        k0, k1 = kind_of(c0), kind_of(c1)
        pri = {"g": 0, "x": 1, "1": 2, "0": 3}
        if pri[k0] > pri[k1]:
            c0, c1, k0, k1 = c1, c0, k1, k0
        kk = k0 + k1
        if "0" in kk:
            gates[i] = {"op": "zero"}
        elif kk == "x1":
            gates[i] = {"op": "copyx", "c": c0}
        elif kk == "11":
            gates[i] = {"op": "one"}
        elif kk == "g1":
            gates[i] = {"op": "g1", "p": c0 - BASE}
        elif kk == "xx":
            gates[i] = {"op": "xx", "a": c0, "b": c1}
        elif kk == "gx":
            gates[i] = {"op": "gx", "p": c0 - BASE, "c": c1}
        elif kk == "gg":
            gates[i] = {"op": "gg", "p": c0 - BASE, "q": c1 - BASE}
        else:
            raise AssertionError(f"unexpected operand kinds {kk}")

    def parents(i):
        d = gates[i]
        return [d[k] for k in ("p", "q") if k in d]

    level = {}
    for i in range(N_GATES):
        ps = parents(i)
        level[i] = (1 + max(level[p] for p in ps)) if ps else 0
    max_level = max(level.values())

    consumers = {i: [] for i in range(N_GATES)}
    for i in range(N_GATES):
        for p in parents(i):
            consumers[p].append(i)
    consumed = {i for i in range(N_GATES) if consumers[i]}

    lv_gates = {l: [i for i in range(N_GATES) if level[i] == l]
                for l in range(max_level + 1)}

    # ---- slot assignment -------------------------------------------------
    # L0: xx gates ordered by first consumer (so the next level's parent
    # slots ascend stride-1), then copyx, then zero/one (adjacent memsets).
    # For gg consumers, both parents share the consumer key and land
    # adjacent (tiebreak: p before q), giving dp=dq=2 runs.
    def fc_key(i):
        cons = [(level[j], j, 0 if gates[j].get("p") == i else 1)
                for j in consumers[i]]
        return (min(cons) if cons else (99, i, 0), i)

    slot_of = {}
    gate_of = {}
    s = 0
    ordkind = {"xx": 0, "copyx": 1, "zero": 2, "one": 2}
    l0 = sorted(lv_gates[0], key=lambda i: (ordkind[gates[i]["op"]], fc_key(i)))
    for i in l0:
        slot_of[i] = s
        s += 1
    # L1+: gx sorted by parent slot, then gg (by parent pair), then g1
    for l in range(1, max_level + 1):
        def key(i):
            d = gates[i]
            ps = sorted(slot_of[d[k]] for k in ("p", "q") if k in d)
            return ({"gx": 0, "gg": 1, "g1": 2}[d["op"]], ps[0], ps[-1], i)
        for i in sorted(lv_gates[l], key=key):
            slot_of[i] = s
            s += 1
    assert s == N_GATES
    for i, s2 in slot_of.items():
        gate_of[s2] = i

    # ---- column placement + ops -----------------------------------------
    colmap = []
    col_pos = {}

    def place(c, fresh=False):
        if not fresh and c in col_pos:
            return col_pos[c]
        pos = len(colmap)
        colmap.append(c)
        if c not in col_pos:
            col_pos[c] = pos
        return pos

    ops = []
    corrected = set()             # slots storing g = 1-m

    for l in range(max_level + 1):
        gl = sorted(lv_gates[l], key=lambda i: slot_of[i])

        idx = 0
        while idx < len(gl):
            i = gl[idx]
            d = gates[i]
            op = d["op"]
            if op == "zero":
                ops.append({"k": "memset", "s0": slot_of[i], "n": 1,
                            "val": 0.0})
                idx += 1
            elif op == "one":
                ops.append({"k": "memset", "s0": slot_of[i], "n": 1,
                            "val": 1.0})
                idx += 1
            elif op == "copyx":
                run = [i]
                jdx = idx + 1
                while jdx < len(gl) and gates[gl[jdx]]["op"] == "copyx":
                    run.append(gl[jdx])
                    jdx += 1
                c0 = None
                for g2 in run:
                    pos = place(gates[g2]["c"], fresh=len(run) > 1)
                    c0 = pos if c0 is None else c0
                ops.append({"k": "copyx", "s0": slot_of[run[0]],
                            "n": len(run), "c0": c0})
                idx = jdx
            elif op == "g1":
                ops.append({"k": "copyv", "s0": slot_of[i], "n": 1,
                            "p0": slot_of[d["p"]]})
                idx += 1
            elif op == "xx":
                # one batched op: A then B regions, every ref placed fresh
                run = [i]
                jdx = idx + 1
                while jdx < len(gl) and gates[gl[jdx]]["op"] == "xx":
                    run.append(gl[jdx])
                    jdx += 1
                if len(run) >= 2:
                    a0 = len(colmap)
                    for g2 in run:
                        place(gates[g2]["a"], fresh=True)
                    b0 = len(colmap)
                    for g2 in run:
                        place(gates[g2]["b"], fresh=True)
                    ops.append({"k": "tt", "s0": slot_of[run[0]], "ds": 1,
                                "n": len(run),
                                "in0": ("x", a0, 1), "in1": ("x", b0, 1)})
                else:
                    ops.append({"k": "tt", "s0": slot_of[i], "ds": 1, "n": 1,
                                "in0": ("x", place(d["a"]), 1),
                                "in1": ("x", place(d["b"]), 1)})
                idx = jdx
            elif op == "gx":
                # run: const parent stride; cols dup-placed to keep going
                run = [i]
                jdx = idx + 1
                dstride = None
                while jdx < len(gl):
                    nd = gates[gl[jdx]]
                    if nd["op"] != "gx":
                        break
                    step = slot_of[nd["p"]] - slot_of[gates[run[-1]]["p"]]
                    if step <= 0 or (dstride is not None and step != dstride):
                        break
                    dstride = step
                    run.append(gl[jdx])
                    jdx += 1
                if len(run) >= 2:
                    c0 = None
                    for g2 in run:
                        pos = place(gates[g2]["c"], fresh=True)
                        c0 = pos if c0 is None else c0
                    ops.append({"k": "tt", "s0": slot_of[run[0]], "ds": 1,
                                "n": len(run),
                                "in0": ("v", slot_of[d["p"]], dstride),
                                "in1": ("x", c0, 1)})
                    idx = jdx
                else:
                    ops.append({"k": "tt", "s0": slot_of[i], "ds": 1, "n": 1,
                                "in0": ("v", slot_of[d["p"]], 1),
                                "in1": ("x", place(d["c"]), 1)})
                    idx += 1
            elif op == "gg":
                run = [i]
                jdx = idx + 1
                dp = dq = None
                while jdx < len(gl):
                    nd = gates[gl[jdx]]
                    if nd["op"] != "gg":
                        break
                    sp = slot_of[nd["p"]] - slot_of[gates[run[-1]]["p"]]
                    sq = slot_of[nd["q"]] - slot_of[gates[run[-1]]["q"]]
                    if sp <= 0 or sq <= 0:
                        break
                    if dp is not None and (sp != dp or sq != dq):
                        break
                    dp, dq = sp, sq
                    run.append(gl[jdx])
                    jdx += 1
                if len(run) >= 2:
                    ops.append({"k": "tt", "s0": slot_of[run[0]], "ds": 1,
                                "n": len(run),
                                "in0": ("v", slot_of[d["p"]], dp),
                                "in1": ("v", slot_of[d["q"]], dq)})
                    idx = jdx
                else:
                    ops.append({"k": "tt", "s0": slot_of[i], "ds": 1, "n": 1,
                                "in0": ("v", slot_of[d["p"]], 1),
                                "in1": ("v", slot_of[d["q"]], 1)})
                    idx += 1
            else:
                raise AssertionError(op)

        # corrections: runs over consumed slots, bridging gaps <= 3
        cons = sorted(slot_of[i] for i in gl if i in consumed)
        while cons:
            lo = hi = cons.pop(0)
            while cons and cons[0] - hi <= 3:
                hi = cons.pop(0)
            ops.append({"k": "corr", "s0": lo, "n": hi - lo + 1})
            corrected.update(range(lo, hi + 1))

    # ---- projection constants -------------------------------------------
    wsig = np.zeros((N_GATES, N_OUTPUTS))
    bias = np.zeros(N_OUTPUTS)
    for s2 in range(N_GATES):
        wrow = W[gate_of[s2]] * scale
        if s2 in corrected:
            wsig[s2] = wrow
        else:
            wsig[s2] = -wrow
            bias += wrow

    # PE path rows: r = t'*64 + g
    wt = np.zeros((P, 2 * N_OUTPUTS))
    for tp in range(2):
        wt[tp * N_GATES:(tp + 1) * N_GATES,
           tp * N_OUTPUTS:(tp + 1) * N_OUTPUTS] = wsig
    # XBAR path rows: r = 2g + u
    wu = np.zeros((P, 2 * N_OUTPUTS))
    for u in range(2):
        wu[u::2, u * N_OUTPUTS:(u + 1) * N_OUTPUTS] = wsig

    return {
        "gates": gates,
        "ops": ops,
        "colmap": np.array(colmap, dtype=np.int64),
        "ncols": len(colmap),
        "slot_of": slot_of,
        "gate_of": gate_of,
        "consumed": consumed,
        "corrected": corrected,
        "level": level,
        "wt_bf16": wt.astype(ml_dtypes.bfloat16),
        "wu_bf16": wu.astype(ml_dtypes.bfloat16),
        "bias_f32": bias.astype(np.float32),
    }


def _emulate_vt(plan, xt):
    """Emulate the device gate program on sample-major xt [n, ncols] bf16.
    Returns V [n, 64] bf16 slot values."""
    bf = ml_dtypes.bfloat16
    n = xt.shape[0]
    V = np.zeros((n, N_GATES), dtype=bf)
    for op in plan["ops"]:
        k, s0, nn = op["k"], op["s0"], op["n"]
        if k == "memset":
            V[:, s0:s0 + nn] = op["val"]
        elif k == "copyx":
            V[:, s0:s0 + nn] = xt[:, op["c0"]:op["c0"] + nn]
        elif k == "copyv":
            V[:, s0:s0 + nn] = V[:, op["p0"]:op["p0"] + nn]
        elif k == "tt":
            def rd(spec):
                src, o0, ds = spec
                arr = xt if src == "x" else V
                return arr[:, o0:o0 + ds * nn:ds].astype(np.float32)
            V[:, s0:s0 + nn] = (rd(op["in0"]) * rd(op["in1"])).astype(bf)
        elif k == "corr":
            m = V[:, s0:s0 + nn].astype(np.float32)
            V[:, s0:s0 + nn] = (1.0 - m).astype(bf)
        else:
            raise AssertionError(k)
    return V


def _schedule():
    """Derive per-chunk unit lists. Unit k covers pairs [8k, 8k+8)."""
    sched = []
    u0 = 0
    for ci, cp in enumerate(CHUNKS):
        nu = cp // 8
        xb = XB_UNITS[ci]
        sched.append({
            "t0": sum(CHUNKS[:ci]),
            "tn": cp,
            "units": list(range(u0, u0 + nu)),
            "pe_units": list(range(u0, u0 + nu - xb)),
            "xb_units": list(range(u0 + nu - xb, u0 + nu)),
        })
        u0 += nu
    return sched


def _build_bass_kernel(plan):
    import concourse.bacc as bacc
    import concourse.tile as tile
    import concourse.mybir as mybir
    from concourse import masks

    f32 = mybir.dt.float32
    bf16 = mybir.dt.bfloat16
    mult = mybir.AluOpType.mult
    add = mybir.AluOpType.add

    C = plan["ncols"]
    sched = _schedule()

    nc = bacc.Bacc(None, target_bir_lowering=False)
    x_d = nc.dram_tensor("xg", [P, NT, C, 2], bf16, kind="ExternalInput")
    wt_d = nc.dram_tensor("wt", [P, 2 * N_OUTPUTS], bf16, kind="ExternalInput")
    wu_d = nc.dram_tensor("wu", [P, 2 * N_OUTPUTS], bf16, kind="ExternalInput")
    out_d = nc.dram_tensor("out", [N_GROUPS, P, 1024], bf16,
                           kind="ExternalOutput")

    with tile.TileContext(nc) as tc:
        with (
            tc.tile_pool(name="const", bufs=1) as cpool,
            tc.tile_pool(name="xp", bufs=1) as xpool,
            tc.tile_pool(name="vp", bufs=3) as vpool,
            tc.tile_pool(name="txp", bufs=2) as txpool,
            tc.tile_pool(name="vsp", bufs=4) as vspool,
            tc.tile_pool(name="stgp", bufs=2) as stgpool,
            tc.tile_pool(name="ptp", bufs=2, space="PSUM") as ptpool,
            tc.tile_pool(name="pop", bufs=3, space="PSUM") as popool,
        ):
            ident = cpool.tile([128, 128], f32)
            masks.make_identity(nc, ident[:])
            wt_sb = cpool.tile([P, 2 * N_OUTPUTS], bf16)
            nc.sync.dma_start(wt_sb[:], wt_d[:])
            wu_sb = cpool.tile([P, 2 * N_OUTPUTS], bf16)
            nc.sync.dma_start(wu_sb[:], wu_d[:])

            xt = xpool.tile([P, NT, C, 2], bf16)
            tsl = NT // IN_SLICES
            for si in range(IN_SLICES):
                nc.sync.dma_start(xt[:, si * tsl:(si + 1) * tsl, :, :],
                                  x_d[:, si * tsl:(si + 1) * tsl, :, :])

            po_tiles = {}
            po_filled = {}
            ndrain = 0

            for ci, sc in enumerate(sched):
                t0, tn = sc["t0"], sc["tn"]
                vt = vpool.tile([P, tn, N_GATES, 2], bf16, tag="vt")

                def vsl(s0, ds, n):
                    if ds == 1:
                        return vt[:, :, s0:s0 + n, :]
                    return vt[:, :, s0:s0 + ds * n:ds, :]

                def xsl(c0, dc, n):
                    if dc == 1:
                        return xt[:, t0:t0 + tn, c0:c0 + n, :]
                    return xt[:, t0:t0 + tn, c0:c0 + dc * n:dc, :]

                for op in plan["ops"]:
                    k, s0, n = op["k"], op["s0"], op["n"]
                    if k == "memset":
                        nc.gpsimd.memset(vsl(s0, 1, n), op["val"])
                    elif k == "copyx":
                        nc.vector.tensor_copy(vsl(s0, 1, n),
                                              xsl(op["c0"], 1, n))
                    elif k == "copyv":
                        nc.vector.tensor_copy(vsl(s0, 1, n),
                                              vsl(op["p0"], 1, n))
                    elif k == "tt":
                        def rd(spec):
                            src, o0, ds = spec
                            return (xsl(o0, ds, n) if src == "x"
                                    else vsl(o0, ds, n))
                        nc.vector.tensor_tensor(
                            vsl(s0, op["ds"], n), rd(op["in0"]),
                            rd(op["in1"]), mult)
                    elif k == "corr":
                        nc.vector.tensor_scalar(
                            vsl(s0, 1, n), vsl(s0, 1, n), -1.0, 1.0,
                            mult, add)
                    else:
                        raise AssertionError(k)

                # ---- output phase ----
                if sc["xb_units"]:
                    xb_p0 = (sc["xb_units"][0] * 8 - t0)   # local pair
                    xb_np = len(sc["xb_units"]) * 8
                    txt = txpool.tile([P, xb_np, P], bf16, tag="txt")
                    nc.sync.dma_start_transpose(
                        txt[:], vt[:, xb_p0:xb_p0 + xb_np, :, :])

                def get_po(k):
                    g = k // 4
                    if g not in po_tiles:
                        po_tiles[g] = popool.tile([P, 1024], f32, name="po",
                                                  tag="po")
                        po_filled[g] = 0
                    return po_tiles[g]

                vb = vt[:].bitcast(f32).rearrange("p t g z -> p (t g z)")
                for k in sc["pe_units"]:
                    po = get_po(k)
                    pt = ptpool.tile([P, 512], f32, tag="pt")
                    for kk in range(4):
                        k2 = (8 * k - t0) // 2 + kk     # local pair-pair
                        nc.tensor.transpose(
                            pt[:, 128 * kk:128 * (kk + 1)],
                            vb[:, 128 * k2:128 * (k2 + 1)], ident[:])
                    vs = vspool.tile([P, 1024], bf16, tag="vs")
                    # GPSIMD cannot access PSUM; drains go Act:DVE 3:1
                    if ndrain % 4 < 3:
                        nc.scalar.copy(vs[:], pt[:].bitcast(bf16))
                    else:
                        nc.vector.tensor_copy(vs[:], pt[:].bitcast(bf16))
                    ndrain += 1
                    m = k % 4
                    for h in range(2):
                        nc.tensor.matmul(
                            po[32 * m:32 * m + 16, 512 * h:512 * h + 512],
                            wt_sb[:], vs[:, 512 * h:512 * h + 512],
                            start=True, stop=True, tile_position=(0, 32 * m))
                    po_filled[k // 4] += 1

                for k in sc["xb_units"]:
                    po = get_po(k)
                    lp = 8 * k - sc["xb_units"][0] * 8
                    m = k % 4
                    for h in range(2):
                        rhs = txt[:, lp + 4 * h:lp + 4 * h + 4, :]
                        nc.tensor.matmul(
                            po[32 * m:32 * m + 16, 512 * h:512 * h + 512],
                            wu_sb[:], rhs.rearrange("r k p -> r (k p)"),
                            start=True, stop=True, tile_position=(0, 32 * m))
                    po_filled[k // 4] += 1

                for g in sorted(po_tiles):
                    if po_filled[g] == 4:
                        stg = stgpool.tile([P, 1024], bf16, tag="stg")
                        nc.scalar.copy(stg[:], po_tiles[g][:])
                        nc.sync.dma_start(out_d[g], stg[:])
                        del po_tiles[g]
                        po_filled[g] = -1

    nc.compile()
    return nc


def _decode_idx():
    """dst[g, r, q] -> flat index into [N_LOC, 8]."""
    sched = _schedule()
    is_xb = {}
    xb_base = {}
    for sc in sched:
        for k in sc["pe_units"]:
            is_xb[k] = False
        for k in sc["xb_units"]:
            is_xb[k] = True
            xb_base[k] = sc["xb_units"][0]
    dst = np.full((N_GROUPS, P, 1024), -1, dtype=np.int64)
    for k in range(N_UNITS):
        g, m = k // 4, k % 4
        for rr in range(16):
            half, o = rr // N_OUTPUTS, rr % N_OUTPUTS
            r = 32 * m + rr
            if not is_xb[k]:
                kk = np.arange(4)[:, None, None]
                p_ = np.arange(P)[None, :, None]
                u = np.arange(2)[None, None, :]
                q = (kk * 256 + p_ * 2 + u).reshape(-1)
                j = (2 * (8 * k + 2 * kk + half) + u + 0 * p_).reshape(-1)
                p_f = (0 * kk + p_ + 0 * u).reshape(-1)
            else:
                tl = np.arange(8)[:, None]
                p_ = np.arange(P)[None, :]
                q = (tl * 128 + p_).reshape(-1)
                j = (2 * (8 * k + tl) + half + 0 * p_).reshape(-1)
                p_f = (0 * tl + p_).reshape(-1)
            dst[g, r, q] = (p_f * 512 + j) * N_OUTPUTS + o
    return dst


_DST = None


def _decode_out(dev_out, plan):
    """[N_GROUPS, P, 1024] bf16 device output -> [N_LOC, 8] f32 w/ bias."""
    global _DST
    if _DST is None:
        _DST = _decode_idx()
    flat = np.empty(N_LOC * N_OUTPUTS, dtype=np.float32)
    mask = _DST.reshape(-1) >= 0
    flat[_DST.reshape(-1)[mask]] = np.asarray(dev_out).astype(
        np.float32).reshape(-1)[mask]
    out = flat.reshape(N_LOC, N_OUTPUTS)
    out += plan["bias_f32"][None, :]
    return out


def make_in_maps(X, plan):
    colmap = plan["colmap"]
    # xg[core][p, t, c, u] = X[core*N_LOC + p*512 + 2t + u, colmap[c]]
    arr = np.asarray(X, dtype=np.float32).reshape(N_CORES, P, NT, 2, N_FEATURES)
    xg = arr[..., colmap].transpose(0, 1, 2, 4, 3).astype(ml_dtypes.bfloat16)
    in_maps = []
    for c in range(N_CORES):
        in_maps.append({
            "xg": np.ascontiguousarray(xg[c]),
            "wt": plan["wt_bf16"],
            "wu": plan["wu_bf16"],
        })
    return in_maps


_CACHE = {}


def _get_compiled(gate_weights, output_weights, output_scale):
    key = hashlib.sha256(
        np.asarray(gate_weights, np.float32).tobytes()
        + np.asarray(output_weights, np.float32).tobytes()
        + np.asarray(output_scale, np.float32).tobytes()
    ).hexdigest()
    if key not in _CACHE:
        plan = _build_plan(gate_weights, output_weights, output_scale)
        nc = _build_bass_kernel(plan)
        _CACHE[key] = (plan, nc)
    return _CACHE[key]


def kernel(X, gate_weights, output_weights, output_scale):
    X = np.asarray(X, dtype=np.float32)
    plan, nc = _get_compiled(gate_weights, output_weights, output_scale)
    in_maps = make_in_maps(X, plan)

    from concourse.bass_utils import run_bass_kernel_spmd
    res = run_bass_kernel_spmd(nc, in_maps, list(range(N_CORES)))
    out = np.concatenate(
        [_decode_out(res.results[c]["out"], plan) for c in range(N_CORES)],
        axis=0)
    return out.astype(np.float32)
